# revision 11
# baseline (speedup 1.0000x reference)
"""Trainium2 Bass kernel for ConvEncoderND (SetConv encoder + pointwise MLP).

Math (per batch element b):
    D[i,o]   = || x_grid[o] - x_context[i] ||                (n_in x n_out)
    E_c[i,o] = exp(-0.5 * D[i,o] / exp(sigma_c)^2)           c in {0,1}
    dens[o]  = sum_i E_0[i,o]
    conv[o]  = sum_i y_context[i] * E_1[i,o]
    out[k,o] = sigmoid(W[k,0]*dens[o] + W[k,1]*conv[o]/(dens[o]+1e-8) + b[k])

Device mapping (one batch element per NeuronCore, 8 cores):
  stage 1 (PE):  D^2 tile = augmented rank-4 matmul
                 rows(lhsT A) = [-2*xc0, -2*xc1, 1, |xc|^2 + eps]
                 rows(rhs  R) = [xg0, xg1, |xg|^2, 1]
  sqrt (ACT, sqrt table set):  D = sqrt(D^2)   PSUM -> SBUF, batched
  exp  (ACT, exp  table set):  E = exp(a * D)  -> bf16, batched after all sqrts
  stage 2 (PE):  [dens; conv] = Y2^T @ E   accumulated over n_in chunks
  normalize (DVE, reshaped to [128,16] per o-half so all lanes are used)
  stage 3 (PE):  z = WB^T @ [dens; convn; 1]
  sigmoid via tanh (tanh lives in the exp table set): 0.5*tanh(0.5*z)+0.5

PSUM is 8 banks; a single shared pool provides 2 slots of 4 banks that are
reused by the D^2 tiles, the stage-2 accumulators (one per o-half) and the
stage-3 logits (one per o-half).
"""

import numpy as np
import ml_dtypes

import concourse.bass as bass
import concourse.tile as tile
from concourse import bacc, mybir
from concourse.bass_utils import run_bass_kernel_spmd
from concourse.tile_rust import add_dep_helper

AF = mybir.ActivationFunctionType
ALU = mybir.AluOpType
F32 = mybir.dt.float32
F32R = mybir.dt.float32r
BF16 = mybir.dt.bfloat16

B = 8
N_IN = 512
N_OUT = 4096
C_OUT = 64
IC = N_IN // 128      # 4 chunks of 128 context points (partition dim)
HW_ = N_OUT // 2      # o-half width (2048 = 4 PSUM banks)
EPSQ = 5e-7           # folded into |xc|^2 so sqrt never sees a negative


def _build_program(a0: float, a1: float, equal_sigma: bool, mm_dtype: str):
    """Build the single-core SPMD bass program. a0/a1 are the exp scales
    (-0.5/scale_c^2) baked in as immediates."""
    nc = bacc.Bacc(
        "TRN2",
        target_bir_lowering=False,
        debug=False,
        num_devices=B,
    )

    A_d = nc.dram_tensor("A", [4, N_IN], F32, kind="ExternalInput")
    R_d = nc.dram_tensor("R", [4, N_OUT], F32, kind="ExternalInput")
    Y2_d = nc.dram_tensor("Y2", [IC, 128, 6], BF16, kind="ExternalInput")
    WB_d = nc.dram_tensor("WB", [3, C_OUT], F32, kind="ExternalInput")
    OUT_d = nc.dram_tensor("OUT", [C_OUT, N_OUT], F32, kind="ExternalOutput")

    n_e = 1 if equal_sigma else 2

    with tile.TileContext(nc) as tc:
        with (
            tc.tile_pool(name="const", bufs=1) as const,
            tc.tile_pool(name="dbuf", bufs=1) as dbuf,
            tc.tile_pool(name="ebuf", bufs=1) as ebuf,
            tc.tile_pool(name="psq", bufs=2, space=bass.MemorySpace.PSUM) as psq,
            tc.tile_pool(name="pst", bufs=1, space=bass.MemorySpace.PSUM) as pst,
        ):
            Asb = const.tile([4, N_IN], F32)
            Rsb = const.tile([4, N_OUT], F32)
            y2sb = const.tile([128, 6 * IC], BF16)
            wbsb = const.tile([3, C_OUT], F32)
            v3 = const.tile([3, N_OUT], F32)
            dn = const.tile([128, N_OUT // 128], F32)
            cv = const.tile([128, N_OUT // 128], F32)
            rc = const.tile([128, N_OUT // 128], F32)
            cvn = const.tile([128, N_OUT // 128], F32)
            tout = const.tile([C_OUT, N_OUT], F32)

            nc.sync.dma_start(out=Asb[:], in_=A_d[:])
            nc.sync.dma_start(out=Rsb[:], in_=R_d[:])
            for c in range(IC):
                nc.sync.dma_start(out=y2sb[:, 6 * c : 6 * c + 6], in_=Y2_d[c])
            nc.sync.dma_start(out=wbsb[:], in_=WB_d[:])
            ones_sb = const.tile([1, N_OUT], F32)
            nc.vector.memset(ones_sb[:], 1.0)
            nc.sync.dma_start(out=v3[2:3, :], in_=ones_sb[:])

            if mm_dtype == "f32r":
                A_mm = Asb[:].bitcast(F32R)
                R_mm = Rsb[:].bitcast(F32R)
            else:
                A_mm = Asb[:]
                R_mm = Rsb[:]

            # D buffer: [128, IC * N_OUT]  (i-chunk c lives at cols c*N_OUT..)
            D = dbuf.tile([128, IC * N_OUT], F32)
            # E buffers (bf16): one per exp scale
            Es = [
                ebuf.tile([128, IC * N_OUT], BF16, name=f"E{e}", tag=f"E{e}")
                for e in range(n_e)
            ]

            # ---- stage 1 (PE) + sqrt pass (ACT, sqrt table) ----
            sqrt_insts = []
            QW = 1024  # q tile width: 2 PSUM banks, x2 bufs = 4 banks
            for c in range(IC):
                for h in range(N_OUT // QW):
                    q = psq.tile([128, QW], F32, name=f"q{c}{h}", tag="psq")
                    for j4 in range(QW // 512):
                        o0 = h * QW + j4 * 512
                        nc.tensor.matmul(
                            q[:, j4 * 512 : (j4 + 1) * 512],
                            A_mm[:, c * 128 : (c + 1) * 128],
                            R_mm[:, o0 : o0 + 512],
                            start=True,
                            stop=True,
                        )
                    d_sl = D[:, c * N_OUT + h * QW : c * N_OUT + (h + 1) * QW]
                    sqrt_insts.append(nc.scalar.activation(d_sl, q[:], AF.Sqrt))

            # ---- exp pass (ACT, exp table) -- must come after ALL sqrts ----
            exp_insts = []
            scales = [a0] if equal_sigma else [a0, a1]
            for e, a in enumerate(scales):
                for c in range(IC):
                    d_sl = D[:, c * N_OUT : (c + 1) * N_OUT]
                    e_sl = Es[e][:, c * N_OUT : (c + 1) * N_OUT]
                    exp_insts.append(
                        nc.scalar.activation(e_sl, d_sl, AF.Exp, 0.0, a)
                    )
            for s in sqrt_insts:
                for x in exp_insts:
                    add_dep_helper(x.ins, s.ins, False, "act table phase order")

            # ---- per o-half: stage 2, normalize, stage 3, sigmoid, store ----
            for h in range(2):
                osl = slice(h * HW_, (h + 1) * HW_)
                fsl = slice(h * 16, (h + 1) * 16)  # [128,16] view of this half

                acc = pst.tile([2, HW_], F32, name=f"acc{h}", tag="pst")
                if equal_sigma:
                    for c in range(IC):
                        for j in range(4):
                            nc.tensor.matmul(
                                acc[:, j * 512 : (j + 1) * 512],
                                y2sb[:, 6 * c : 6 * c + 2],
                                Es[0][
                                    :,
                                    c * N_OUT + h * HW_ + j * 512 :
                                    c * N_OUT + h * HW_ + (j + 1) * 512,
                                ],
                                start=(c == 0),
                                stop=(c == IC - 1),
                            )
                else:
                    # row pair [1,0] over E0 accumulates dens into acc row 0;
                    # row pair [0,yc] over E1 accumulates conv into acc row 1.
                    for row in range(2):
                        for c in range(IC):
                            for j in range(4):
                                nc.tensor.matmul(
                                    acc[:, j * 512 : (j + 1) * 512],
                                    y2sb[:, 6 * c + 2 + 2 * row : 6 * c + 4 + 2 * row],
                                    Es[row][
                                        :,
                                        c * N_OUT + h * HW_ + j * 512 :
                                        c * N_OUT + h * HW_ + (j + 1) * 512,
                                    ],
                                    start=(row == 0 and c == 0),
                                    stop=(row == 1 and c == IC - 1),
                                )

                # normalization: evacuate PSUM via DVE (dens lands in v3 row
                # 0), reshape to [128, 16] via SBUF->SBUF DMA for the divide.
                nc.vector.tensor_copy(v3[0:2, osl], acc[0:2, :])
                nc.sync.dma_start(out=dn[:, fsl], in_=v3[0:1, osl])
                nc.sync.dma_start(out=cv[:, fsl], in_=v3[1:2, osl])
                nc.vector.tensor_scalar_add(rc[:, fsl], dn[:, fsl], 1e-8)
                nc.vector.reciprocal(rc[:, fsl], rc[:, fsl])
                nc.vector.tensor_tensor(
                    cvn[:, fsl], cv[:, fsl], rc[:, fsl], ALU.mult
                )
                nc.sync.dma_start(out=v3[1:2, osl], in_=cvn[:, fsl])

                # stage 3 + sigmoid via tanh
                z = pst.tile([C_OUT, HW_], F32, name=f"z{h}", tag="pst")
                for j in range(4):
                    nc.tensor.matmul(
                        z[:, j * 512 : (j + 1) * 512],
                        wbsb[:],
                        v3[:, h * HW_ + j * 512 : h * HW_ + (j + 1) * 512],
                        start=True,
                        stop=True,
                    )
                th = nc.scalar.activation(tout[:, osl], z[:], AF.Tanh, 0.0, 0.5)
                for s in sqrt_insts:
                    add_dep_helper(th.ins, s.ins, False, "act table phase order")
                nc.vector.tensor_scalar(
                    tout[:, osl], tout[:, osl], 0.5, 0.5, ALU.mult, ALU.add
                )
                nc.sync.dma_start(out=OUT_d[:, osl], in_=tout[:, osl])

    nc.compile()
    return nc


def _prep_inputs(x_context, y_context, x_grid, sigma, W, b):
    """Host-side prep: build per-core augmented tensors (all O(n) work)."""
    scales = np.exp(sigma.astype(np.float64))
    a = (-0.5 / scales**2).astype(np.float64)
    a0, a1 = float(a[0]), float(a[1])
    equal_sigma = abs(a0 - a1) <= 1e-9 * max(abs(a0), abs(a1))

    in_maps = []
    for bi in range(B):
        xc = x_context[bi].astype(np.float32)  # (512, 2)
        xg = x_grid[bi].astype(np.float32)     # (4096, 2)
        yc = y_context[bi, :, 0].astype(np.float32)

        cn = (xc[:, 0] ** 2 + xc[:, 1] ** 2 + EPSQ).astype(np.float32)
        gn = (xg[:, 0] ** 2 + xg[:, 1] ** 2).astype(np.float32)
        A = np.stack(
            [-2.0 * xc[:, 0], -2.0 * xc[:, 1], np.ones(N_IN, np.float32), cn]
        ).astype(np.float32)
        R = np.stack(
            [xg[:, 0], xg[:, 1], gn, np.ones(N_OUT, np.float32)]
        ).astype(np.float32)
        ones = np.ones(N_IN, np.float32)
        zero = np.zeros(N_IN, np.float32)
        Y2 = np.stack([ones, yc, ones, zero, zero, yc], axis=-1)
        Y2 = Y2.reshape(IC, 128, 6).astype(ml_dtypes.bfloat16)
        WB = np.stack([W[:, 0], W[:, 1], b]).astype(np.float32)
        in_maps.append({"A": A, "R": R, "Y2": Y2, "WB": WB})
    return in_maps, a0, a1, equal_sigma


_PROGRAM_CACHE = {}


def run_device(inputs, mm_dtype="f32", trace=False):
    """Run the bass kernel; returns (output (B,64,64,64) f32, BassKernelResults)."""
    in_maps, a0, a1, equal_sigma = _prep_inputs(**inputs)
    key = (round(a0, 12), round(a1, 12), equal_sigma, mm_dtype)
    if key not in _PROGRAM_CACHE:
        _PROGRAM_CACHE[key] = _build_program(a0, a1, equal_sigma, mm_dtype)
    nc = _PROGRAM_CACHE[key]
    res = run_bass_kernel_spmd(nc, in_maps, core_ids=list(range(B)), trace=trace)
    out = np.stack([res.results[i]["OUT"] for i in range(B)])
    out = out.reshape(B, C_OUT, 64, 64).astype(np.float32)
    return out, res


def kernel(**inputs) -> np.ndarray:
    out, _ = run_device(inputs, mm_dtype="f32")
    return out
